# revision 1
# baseline (speedup 1.0000x reference)
"""LLaMA layer (B=2, T=1024, D=2048, H=16 GQA-4, F=5632) on 8 trn2 NeuronCores.

v4 sharding: heads tensor-parallel for attention, token-parallel FFN.
  - core c owns q-heads {2c, 2c+1}, kv-head c//2, and token sub-chunks
    [c*128,(c+1)*128) of batch 0 and batch 1 (256 own tokens total).
  - collectives: two AllToAll's (attn out per batch: heads -> token chunks);
    #1 hides under batch-1 compute, #2 under o-proj half 0.
  - FFN: token-sharded (own 256 tokens x full F), bf16 weights streamed.
  - norm1 folded into rope tables + V scale; norm stats via ones-matmul,
    pipelined one token-group ahead; the inv broadcast matmul is emitted
    behind the K-projection so the sqrt/recip chain never stalls PE.
  - pool layout: the x^T region (bx pool) frees before phase C1 so the
    o-proj weight stream + first FFN chunk land during C1/the collectives.
"""

import numpy as np

NC = 8
B, T, D = 2, 1024, 2048
H, HKV, DH = 16, 4, 128
F = 5632
GLOB = B * T            # 2048 tokens, b-major
TOK = GLOB // NC        # 256 own tokens (128 from each batch)
HTOK = TOK // 2         # 128
DT = D // 128           # 16 D-tiles
FT = F // 128           # 44 F-tiles
NG = GLOB // 512        # 4 moving groups of 512 tokens (0,1=b0; 2,3=b1)
EPS = 1e-6
SCL = DH ** -0.5

_CACHE = {}


def _build_program():
    import concourse.bass as bass
    import concourse.mybir as mybir
    import concourse.tile as tile
    from concourse import bacc
    from concourse.masks import make_identity

    F32 = mybir.dt.float32
    F32R = mybir.dt.float32r
    BF16 = mybir.dt.bfloat16
    AF = mybir.ActivationFunctionType

    nc = bacc.Bacc("TRN2", target_bir_lowering=False, debug=False,
                   enable_asserts=False, num_devices=NC)

    # ---- per-core inputs (host pre-sliced / pre-folded) ----
    xT_in = nc.dram_tensor("xT_in", [D, GLOB], BF16, kind="ExternalInput").ap()
    xc_in = nc.dram_tensor("xc_in", [TOK, D], BF16, kind="ExternalInput").ap()
    wqkv = nc.dram_tensor("wqkv", [D, 512], BF16, kind="ExternalInput").ap()
    wo16 = nc.dram_tensor("wo16", [DT, 128, D], BF16, kind="ExternalInput").ap()
    w1f = nc.dram_tensor("w1f", [D, F], BF16, kind="ExternalInput").ap()
    w3f = nc.dram_tensor("w3f", [D, F], BF16, kind="ExternalInput").ap()
    w2f = nc.dram_tensor("w2f", [FT, 128, D], BF16, kind="ExternalInput").ap()
    cscat = nc.dram_tensor("cscat", [128, GLOB], BF16, kind="ExternalInput").ap()
    sccat = nc.dram_tensor("sccat", [128, GLOB], BF16, kind="ExternalInput").ap()
    maskc = nc.dram_tensor("maskc", [128, 4 * 512], BF16,
                           kind="ExternalInput").ap()

    # ---- per-core outputs ----
    yt = nc.dram_tensor("yt", [TOK, D], F32, kind="ExternalOutput").ap()

    RG = [list(range(NC))]
    w1r = w1f.rearrange("(a p) m -> a p m", p=128)
    w3r = w3f.rearrange("(a p) m -> a p m", p=128)

    with tile.TileContext(nc) as tc:
      with tc.tile_pool(name="const", bufs=1) as cp, \
           tc.tile_pool(name="dram", bufs=1, space="DRAM") as dp:
        # constants
        ident = cp.tile([128, 128], F32, name="ident")
        make_identity(nc, ident[:])
        identb = cp.tile([128, 128], BF16, name="identb")
        nc.vector.tensor_copy(identb[:], ident[:])
        ones_c32 = cp.tile([128, 1], F32, name="ones_c32")
        nc.vector.memset(ones_c32[:], 1.0)
        ones_c = cp.tile([128, 1], BF16, name="ones_c")
        nc.vector.tensor_copy(ones_c[:], ones_c32[:])
        ones_r32 = cp.tile([1, 128], F32, name="ones_r32")
        nc.vector.memset(ones_r32[:], 1.0)
        ones_r = cp.tile([1, 128], BF16, name="ones_r")
        nc.vector.tensor_copy(ones_r[:], ones_r32[:])
        ones_rr = cp.tile([1, 128], F32R, name="ones_rr")
        nc.vector.tensor_copy(ones_rr[:], ones_r32[:])
        eps1 = cp.tile([1, 1], F32, name="eps1")
        nc.vector.memset(eps1[:], EPS)
        scd1 = cp.tile([1, 1], F32, name="scd1")
        nc.vector.memset(scd1[:], 1.0 / D)
        scexp = cp.tile([128, 1], F32, name="scexp")
        nc.vector.memset(scexp[:], SCL)
        eps128 = cp.tile([128, 1], F32, name="eps128")
        nc.vector.memset(eps128[:], EPS)
        scd128 = cp.tile([128, 1], F32, name="scd128")
        nc.vector.memset(scd128[:], 1.0 / D)

        # DRAM bounce buffers for the two batch-split AllToAlls
        o_in = [dp.tile([NC, 2 * DH, HTOK], BF16, name=f"o_in{h}")
                for h in range(2)]
        o_out = [dp.tile([NC, 2 * DH, HTOK], BF16, name=f"o_out{h}")
                 for h in range(2)]

        # persistent activations (span C..E)
        with tc.tile_pool(name="resid", bufs=1) as rp:
            oT = [rp.tile([128, GLOB], BF16, name=f"oT{h}") for h in range(2)]
            # o-proj weight stream (two passes over wo16, rotating buffers)
            # + half-0 attn chunks; tiles live here so the loads can be
            # emitted before batch-1 compute
            wos = [rp.tile([128, D], BF16, name=f"wos{i}",
                           tag="wos", bufs=10) for i in range(2 * DT)]
            oT0 = [rp.tile([128, HTOK], BF16, name=f"oTo0_{k}")
                   for k in range(DT)]

            # ======== phases B+C: QKV+rope+attention, per batch ========
            with tc.tile_pool(name="tabs", bufs=1) as tb:
                qT = [tb.tile([128, GLOB], BF16, name=f"qT{h}")
                      for h in range(2)]
                kT = tb.tile([128, GLOB], BF16, name="kT")
                Vn4 = [tb.tile([128, 512], BF16, name=f"Vn4{g}")
                       for g in range(NG)]
                mk = tb.tile([128, 4 * 512], BF16, name="mk")

                with tc.tile_pool(name="phBC", bufs=1) as pb2, \
                     tc.tile_pool(name="psBC", bufs=1, space="PSUM") as psBC:
                    invp = {}
                    csx = {}
                    scx = {}
                    ibs = {}

                    with tc.tile_pool(name="bx", bufs=1) as bxp:
                        wq_sb = []
                        xT = []
                        for k in range(DT):
                            wq_sb.append(bxp.tile([128, 512], BF16,
                                                  name=f"wqkv_sb{k}"))
                            xT.append(bxp.tile([128, GLOB], BF16,
                                               name=f"xT{k}"))
                        cs_cat = bxp.tile([128, GLOB], BF16, name="cs_cat")
                        sc_cat = bxp.tile([128, GLOB], BF16, name="sc_cat")
                        # startup DMA order: xT[k]+wqkv[k] pairs first
                        # (compute is k-paced behind these), tables after.
                        xT_r = xT_in.rearrange("(a p) m -> a p m", p=128)
                        wqkv_r = wqkv.rearrange("(a p) m -> a p m", p=128)
                        for k in range(DT):
                            nc.sync.dma_start(xT[k][:], xT_r[k])
                            nc.sync.dma_start(wq_sb[k][:], wqkv_r[k])
                        nc.sync.dma_start(cs_cat[:], cscat[:])
                        nc.sync.dma_start(sc_cat[:], sccat[:])
                        nc.sync.dma_start(mk[:], maskc[:])

                        def stats_mm(g, k):
                            # accumulate sum of x^2 over D for token group g
                            gc = slice(g * 512, (g + 1) * 512)
                            if k == 0:
                                invp[g] = psBC.tile([1, 512], F32,
                                                    name=f"invp{g}",
                                                    tag="prow", bufs=2)
                            sq = pb2.tile([128, 512], BF16, name=f"sq{g}_{k}",
                                          tag="sq", bufs=2)
                            nc.vector.tensor_mul(sq[:], xT[k][:, gc],
                                                 xT[k][:, gc])
                            nc.tensor.matmul(invp[g][:], ones_c[:], sq[:],
                                             start=(k == 0),
                                             stop=(k == DT - 1))

                        invs = {}

                        def stats_fin_a(g):
                            std = pb2.tile([1, 512], F32, name=f"std{g}",
                                           tag="std", bufs=2)
                            nc.scalar.activation(std[:], invp[g][:], AF.Sqrt,
                                                 scale=scd1[:], bias=eps1[:])
                            inv = pb2.tile([1, 512], F32R, name=f"inv{g}",
                                           tag="inv", bufs=2)
                            with nc.allow_low_precision(reason="norm1 recip"):
                                nc.vector.reciprocal(inv[:], std[:])
                            invs[g] = inv

                        def stats_fin_b(g):
                            gc = slice(g * 512, (g + 1) * 512)
                            ibp = psBC.tile([128, 512], F32, name=f"ibp{g}",
                                            tag="pacc", bufs=3)
                            nc.tensor.matmul(ibp[:], ones_rr[:], invs[g][:],
                                             start=True, stop=True)
                            ib = pb2.tile([128, 512], F32, name=f"ibs{g}",
                                          tag="ibs", bufs=2)
                            nc.scalar.copy(ib[:], ibp[:])
                            ibs[g] = ib
                            cx = pb2.tile([128, 512], F32, name=f"csx{g}",
                                          tag="csx", bufs=2)
                            sx = pb2.tile([128, 512], F32, name=f"scx{g}",
                                          tag="scx", bufs=2)
                            nc.vector.tensor_mul(cx[:], cs_cat[:, gc], ib[:])
                            nc.vector.tensor_mul(sx[:], sc_cat[:, gc], ib[:])
                            csx[g] = cx
                            scx[g] = sx

                        def rope(g, ps, dst, tag):
                            gc = slice(g * 512, (g + 1) * 512)
                            cx, sx = csx[g], scx[g]
                            a = pb2.tile([64, 512], F32, name=f"ra_{tag}",
                                         tag="ra", bufs=1)
                            b_ = pb2.tile([64, 512], F32, name=f"rb_{tag}",
                                          tag="rb", bufs=1)
                            nc.vector.tensor_mul(a[:], ps[0:64, :],
                                                 cx[0:64, :])
                            nc.vector.tensor_mul(b_[:], ps[64:128, :],
                                                 cx[64:128, :])
                            nc.vector.tensor_sub(dst[0:64, gc], a[:], b_[:])
                            c_ = pb2.tile([64, 512], F32, name=f"rc_{tag}",
                                          tag="rc", bufs=1)
                            d_ = pb2.tile([64, 512], F32, name=f"rd_{tag}",
                                          tag="rd", bufs=1)
                            nc.vector.tensor_mul(c_[:], ps[0:64, :],
                                                 sx[0:64, :])
                            nc.vector.tensor_mul(d_[:], ps[64:128, :],
                                                 sx[64:128, :])
                            nc.vector.tensor_add(dst[64:128, gc], c_[:],
                                                 d_[:])

                        def qkv_ps(g, name):
                            return psBC.tile([128, 512], F32,
                                             name=f"ps{name}_{g}",
                                             tag="pmm", bufs=3)

                        def qkv_mm(ps, g, k, c0, c1):
                            gc = slice(g * 512, (g + 1) * 512)
                            nc.tensor.matmul(ps[:], wq_sb[k][:, c0:c1],
                                             xT[k][:, gc], start=(k == 0),
                                             stop=(k == DT - 1))

                        def vfin(g, ps):
                            vTg = pb2.tile([128, 512], BF16, name=f"vTg{g}",
                                           tag="vTg", bufs=2)
                            nc.vector.tensor_mul(vTg[:], ps[:], ibs[g][:])
                            tpV = psBC.tile([128, 512], BF16, name=f"tpV{g}",
                                            tag="pacc", bufs=3)
                            for tt in range(4):
                                nc.tensor.transpose(
                                    tpV[:, tt * 128:(tt + 1) * 128],
                                    vTg[:, tt * 128:(tt + 1) * 128],
                                    identb[:])
                            nc.scalar.copy(Vn4[g][:], tpV[:])

                        def phase_b(g, first=False):
                            # group-g QKV; stats of group g+1 pipelined in.
                            # ibp broadcast is emitted behind the K loop so
                            # its sqrt/recip chain hides under PE work.
                            psq0 = qkv_ps(g, "q0")
                            psq1 = qkv_ps(g, "q1")
                            for k in range(DT):
                                if first:
                                    stats_mm(0, k)
                                qkv_mm(psq0, g, k, 0, 128)
                                qkv_mm(psq1, g, k, 128, 256)
                            if first:
                                stats_fin_a(0)
                            psk = qkv_ps(g, "k")
                            for k in range(DT):
                                qkv_mm(psk, g, k, 256, 384)
                            if first:
                                stats_fin_b(0)
                            rope(g, psq0, qT[0], f"q0_{g}")
                            rope(g, psq1, qT[1], f"q1_{g}")
                            psv = qkv_ps(g, "v")
                            for k in range(DT):
                                qkv_mm(psv, g, k, 384, 512)
                                if g < NG - 1:
                                    stats_mm(g + 1, k)
                            if g < NG - 1:
                                stats_fin_a(g + 1)
                            rope(g, psk, kT, f"k{g}")
                            vfin(g, psv)
                            if g < NG - 1:
                                stats_fin_b(g + 1)

                        def phase_c(b2):
                            for hl in range(2):
                                for qg in range(2):
                                    qc = slice(b2 * T + qg * 512,
                                               b2 * T + (qg + 1) * 512)
                                    nkt = 4 * (qg + 1)
                                    pso = psBC.tile([128, 512], F32,
                                                    name=f"pso{b2}{hl}{qg}",
                                                    tag="pacc", bufs=3)
                                    pssum = psBC.tile(
                                        [1, 512], F32,
                                        name=f"pssum{b2}{hl}{qg}",
                                        tag="prow", bufs=2)
                                    eus = []

                                    def flush_eu(kt):
                                        eu = eus[kt]
                                        nc.tensor.matmul(
                                            pssum[:], ones_c[:], eu[:],
                                            start=(kt == 0),
                                            stop=(kt == nkt - 1))
                                        gt = b2 * 8 + kt
                                        nc.tensor.matmul(
                                            pso[:],
                                            Vn4[gt // 4][:, (gt % 4) * 128:
                                                         (gt % 4 + 1) * 128],
                                            eu[:],
                                            start=(kt == 0),
                                            stop=(kt == nkt - 1))

                                    for kt in range(nkt):
                                        pss = psBC.tile(
                                            [128, 512], F32,
                                            name=f"pss{b2}{hl}{qg}{kt}",
                                            tag="pmm", bufs=3)
                                        k0 = b2 * T + kt * 128
                                        nc.tensor.matmul(
                                            pss[:], kT[:, k0:k0 + 128],
                                            qT[hl][:, qc], start=True,
                                            stop=True)
                                        e = pb2.tile(
                                            [128, 512], BF16,
                                            name=f"e{b2}{hl}{qg}{kt}",
                                            tag="e", bufs=3)
                                        nc.scalar.activation(e[:], pss[:],
                                                             AF.Exp,
                                                             scale=scexp[:])
                                        v = kt - 4 * qg
                                        if 0 <= v <= 3:
                                            em = pb2.tile(
                                                [128, 512], BF16,
                                                name=f"em{b2}{hl}{qg}{kt}",
                                                tag="em", bufs=2)
                                            nc.vector.tensor_mul(
                                                em[:], e[:],
                                                mk[:, v * 512:(v + 1) * 512])
                                            eus.append(em)
                                        else:
                                            eus.append(e)
                                        # one-kt software pipeline: the exp
                                        # latency of kt hides under pss(kt+1)
                                        if kt > 0:
                                            flush_eu(kt - 1)
                                    flush_eu(nkt - 1)
                                    rec = pb2.tile([1, 512], F32R,
                                                   name=f"rec{b2}{hl}{qg}",
                                                   tag="rec", bufs=2)
                                    with nc.allow_low_precision(
                                            reason="softmax recip"):
                                        nc.vector.reciprocal(rec[:],
                                                             pssum[:])
                                    rbc = psBC.tile([128, 512], F32,
                                                    name=f"rbc{b2}{hl}{qg}",
                                                    tag="pacc", bufs=3)
                                    nc.tensor.matmul(rbc[:], ones_rr[:],
                                                     rec[:], start=True,
                                                     stop=True)
                                    rbs = pb2.tile([128, 512], F32,
                                                   name=f"rbs{b2}{hl}{qg}",
                                                   tag="rbs", bufs=2)
                                    nc.scalar.copy(rbs[:], rbc[:])
                                    nc.vector.tensor_mul(oT[hl][:, qc],
                                                         pso[:], rbs[:])

                        phase_b(0, first=True)
                        phase_b(1)
                        phase_c(0)
                        # send batch-0 attn out (split across both queues);
                        # collective #1 overlaps batch-1 compute
                        for hl in range(2):
                            for j in range(NC):
                                t0 = j * HTOK
                                eng = nc.sync if j % 2 == 0 else nc.scalar
                                eng.dma_start(
                                    o_in[0][j, hl * 128:(hl + 1) * 128, :],
                                    oT[hl][:, t0:t0 + HTOK])
                        nc.gpsimd.collective_compute(
                            "AllToAll", mybir.AluOpType.bypass,
                            replica_groups=RG,
                            ins=[o_in[0][:]], outs=[o_out[0][:]])
                        # early-emitted D-phase loads: their conservative
                        # PE-count guards now resolve at C0, so the
                        # streams run during batch-1 compute
                        for i in range(2 * DT):
                            nc.sync.dma_start(wos[i][:], wo16[i % DT])
                        oo0 = o_out[0].rearrange(
                            "r (a p) t -> (r a) p t", p=128)
                        for k in range(DT):
                            nc.sync.dma_start(oT0[k][:], oo0[k])
                        phase_b(2)
                        phase_b(3)
                    # bx pool (x^T / qkv weights / rope tables) closes here:
                    # its SBUF frees for the D-phase streams during C1.
                    phase_c(1)

            # ======== phase D: o-proj + residual + norm2 ========
            with tc.tile_pool(name="pre", bufs=1) as prep:
              # first FFN chunk, prefetched during C1/collectives
              w1c0 = prep.tile([128, DT * 512], BF16, name="w1c0")
              w3c0 = prep.tile([128, DT * 512], BF16, name="w3c0")
              with tc.tile_pool(name="late", bufs=1) as lp:
                hT = [lp.tile([128, TOK], BF16, name=f"hT{d}")
                      for d in range(DT)]
                x1row = [lp.tile([128, D], F32, name=f"x1row{h}")
                         for h in range(2)]
                with tc.tile_pool(name="phD", bufs=1) as pd, \
                     tc.tile_pool(name="psD", bufs=1, space="PSUM") as psD:
                    # residual rows (the wo stream + half-0 chunks were
                    # loaded during batch-1 compute)
                    xcr = []
                    for h in range(2):
                        t_ = pd.tile([128, D], BF16, name=f"xcr{h}")
                        nc.sync.dma_start(t_[:],
                                          xc_in[h * 128:(h + 1) * 128, :])
                        xcr.append(t_)
                    def dhalf(h, oTo, wosl):
                        # o-proj in [tok, d]: stationary = received attn
                        # chunk, moving = wo tiles; psum outputs are split
                        # into bank-sized 512-column groups (ISA limit)
                        p2 = [psD.tile([128, 512], F32, name=f"p2_{h}{j}",
                                       tag="p2", bufs=4) for j in range(4)]
                        for k in range(DT):
                            for j in range(4):
                                nc.tensor.matmul(
                                    p2[j][:], oTo[k][:],
                                    wosl[k][:, j * 512:(j + 1) * 512],
                                    start=(k == 0), stop=(k == DT - 1))
                        for j in range(4):
                            js = slice(j * 512, (j + 1) * 512)
                            nc.vector.tensor_add(x1row[h][:, js], p2[j][:],
                                                 xcr[h][:, js])

                    def norm2half(h):
                        # norm2 stats row-wise on the Act engine
                        scr = pd.tile([128, D // 2], BF16, name=f"scr{h}",
                                      tag="scr", bufs=2)
                        sa = pd.tile([128, 1], F32, name=f"sa{h}",
                                     tag="sa", bufs=2)
                        sb_ = pd.tile([128, 1], F32, name=f"sb{h}",
                                      tag="sb", bufs=2)
                        nc.scalar.activation(scr[:], x1row[h][:, 0:D // 2],
                                             AF.Square, accum_out=sa[:])
                        nc.scalar.activation(scr[:], x1row[h][:, D // 2:D],
                                             AF.Square, accum_out=sb_[:])
                        ssq = pd.tile([128, 1], F32, name=f"ssq{h}",
                                      tag="ssqh", bufs=2)
                        nc.vector.tensor_add(ssq[:], sa[:], sb_[:])
                        st2 = pd.tile([128, 1], F32, name=f"std2_{h}",
                                      tag="st2", bufs=2)
                        nc.scalar.activation(st2[:], ssq[:], AF.Sqrt,
                                             scale=scd128[:], bias=eps128[:])
                        iv = pd.tile([128, 1], F32, name=f"inv2_{h}",
                                     tag="iv", bufs=2)
                        nc.vector.reciprocal(iv[:], st2[:])
                        hr = pd.tile([128, D], BF16, name=f"hrow{h}",
                                     tag="hrow", bufs=2)
                        for q in range(4):
                            qs = slice(q * (D // 4), (q + 1) * (D // 4))
                            nc.scalar.activation(hr[:, qs], x1row[h][:, qs],
                                                 AF.Copy, scale=iv[:])
                        return hr

                    # half 0 runs while collective #1 is in flight; its
                    # sends go out on the scalar queue first so the
                    # collective starts right after batch-1 attention.
                    for hl in range(2):
                        for j in range(NC):
                            t0 = T + j * HTOK
                            nc.scalar.dma_start(
                                o_in[1][j, hl * 128:(hl + 1) * 128, :],
                                oT[hl][:, t0:t0 + HTOK])
                    nc.gpsimd.collective_compute(
                        "AllToAll", mybir.AluOpType.bypass, replica_groups=RG,
                        ins=[o_in[1][:]], outs=[o_out[1][:]])
                    dhalf(0, oT0, wos[0:DT])
                    hrow0 = norm2half(0)
                    # first FFN chunk loads after D-h0 is emitted (the
                    # conservative PE-count guards would otherwise gate
                    # D-h0 behind these loads)
                    nc.sync.dma_start(
                        w1c0[:].rearrange("p (a m) -> p a m", a=DT),
                        w1r[:, :, 0:512].rearrange("a p m -> p a m"))
                    nc.sync.dma_start(
                        w3c0[:].rearrange("p (a m) -> p a m", a=DT),
                        w3r[:, :, 0:512].rearrange("a p m -> p a m"))
                    oo1 = o_out[1].rearrange("r (a p) t -> (r a) p t", p=128)
                    oT1 = []
                    for k in range(DT):
                        t_ = pd.tile([128, HTOK], BF16, name=f"oTo1_{k}")
                        nc.scalar.dma_start(t_[:], oo1[k])
                        oT1.append(t_)
                    dhalf(1, oT1, wos[DT:2 * DT])
                    hrow1 = norm2half(1)
                    hrow = [hrow0, hrow1]
                    # transpose h back to [d, tok] tiles for the FFN
                    for dk in range(DT):
                        tp2 = psD.tile([128, TOK], BF16, name=f"tp2_{dk}",
                                       tag="tp2", bufs=2)
                        for h in range(2):
                            nc.tensor.transpose(
                                tp2[:, h * 128:(h + 1) * 128],
                                hrow[h][:, dk * 128:(dk + 1) * 128],
                                identb[:])
                        nc.scalar.copy(hT[dk][:], tp2[:])

                # ======== phase E: FFN token-sharded, weights streamed ====
                with tc.tile_pool(name="phE", bufs=1) as pe:
                    zT = [pe.tile([128, TOK], BF16, name=f"zT{f}")
                          for f in range(FT)]
                    with tc.tile_pool(name="phEa", bufs=1) as pea, \
                         tc.tile_pool(name="psEa", bufs=1,
                                      space="PSUM") as psEa:
                        for fc in range(11):
                            if fc == 2:
                                nc.sync.dma_start(
                                    w1c0[:, 0:2 * D].rearrange(
                                        "p (a m) -> p a m", a=2),
                                    w2f[0:2].rearrange("a p m -> p a m"))
                                nc.sync.dma_start(
                                    w3c0[:, 0:2 * D].rearrange(
                                        "p (a m) -> p a m", a=2),
                                    w2f[2:4].rearrange("a p m -> p a m"))
                            f0 = fc * 512
                            if fc == 0:
                                w1c, w3c = w1c0, w3c0
                            else:
                                w1c = pea.tile([128, DT * 512], BF16,
                                               name=f"w1c{fc}", tag="w1c",
                                               bufs=2)
                                nc.sync.dma_start(
                                    w1c[:].rearrange("p (a m) -> p a m",
                                                     a=DT),
                                    w1r[:, :, f0:f0 + 512].rearrange(
                                        "a p m -> p a m"))
                                w3c = pea.tile([128, DT * 512], BF16,
                                               name=f"w3c{fc}", tag="w3c",
                                               bufs=2)
                                nc.sync.dma_start(
                                    w3c[:].rearrange("p (a m) -> p a m",
                                                     a=DT),
                                    w3r[:, :, f0:f0 + 512].rearrange(
                                        "a p m -> p a m"))
                            for fi in range(4):
                                ft = fc * 4 + fi
                                pg = psEa.tile([128, TOK], F32,
                                               name=f"pg{ft}", tag="pg",
                                               bufs=2)
                                for k in range(DT):
                                    nc.tensor.matmul(
                                        pg[:],
                                        w1c[:, k * 512 + fi * 128:
                                            k * 512 + (fi + 1) * 128],
                                        hT[k][:], start=(k == 0),
                                        stop=(k == DT - 1))
                                pu = psEa.tile([128, TOK], F32,
                                               name=f"pu{ft}", tag="pu",
                                               bufs=2)
                                for k in range(DT):
                                    nc.tensor.matmul(
                                        pu[:],
                                        w3c[:, k * 512 + fi * 128:
                                            k * 512 + (fi + 1) * 128],
                                        hT[k][:], start=(k == 0),
                                        stop=(k == DT - 1))
                                sil = pea.tile([128, TOK], BF16,
                                               name=f"sil{ft}", tag="sil",
                                               bufs=3)
                                nc.scalar.activation(sil[:], pg[:], AF.Silu)
                                nc.vector.tensor_mul(zT[ft][:], sil[:],
                                                     pu[:])
                    with tc.tile_pool(name="phEb", bufs=1) as peb, \
                         tc.tile_pool(name="psEb", bufs=1,
                                      space="PSUM") as psEb:
                        pf = [[psEb.tile([128, 512], F32,
                                         name=f"pf{h}{j}", tag="pf", bufs=8)
                               for j in range(4)] for h in range(2)]
                        NP = FT // 2
                        w2ts = {0: w1c0, 1: w3c0}

                        def w2load(fp):
                            w2t = peb.tile([128, 2 * D], BF16,
                                           name=f"w2t{fp}", tag="w2t", bufs=4)
                            nc.sync.dma_start(
                                w2t[:].rearrange("p (a m) -> p a m", a=2),
                                w2f[2 * fp:2 * fp + 2].rearrange(
                                    "a p m -> p a m"))
                            w2ts[fp] = w2t

                        w2load(2)
                        w2load(3)
                        for fp in range(NP):
                            if 2 <= fp and fp + 2 < NP:
                                w2load(fp + 2)
                            w2t = w2ts.pop(fp)
                            for fi in range(2):
                                ft = 2 * fp + fi
                                for h in range(2):
                                    for j in range(4):
                                        nc.tensor.matmul(
                                            pf[h][j][:],
                                            zT[ft][:, h * HTOK:
                                               (h + 1) * HTOK],
                                            w2t[:, fi * D + j * 512:
                                                fi * D + (j + 1) * 512],
                                            start=(ft == 0),
                                            stop=(ft == FT - 1))
                        for h in range(2):
                            fo = peb.tile([128, D], F32, name=f"fo{h}",
                                          tag="fo", bufs=2)
                            for j in range(4):
                                js = slice(j * 512, (j + 1) * 512)
                                nc.vector.tensor_add(fo[:, js], pf[h][j][:],
                                                     x1row[h][:, js])
                            nc.scalar.dma_start(
                                yt[h * 128:(h + 1) * 128, :], fo[:])
    nc.compile()
    return nc


def _prep_inputs(inputs):
    import ml_dtypes
    BF = ml_dtypes.bfloat16

    x = np.asarray(inputs["x"], np.float32)
    cos = np.asarray(inputs["freqs_cos"], np.float32)
    sin = np.asarray(inputs["freqs_sin"], np.float32)
    wn1 = np.asarray(inputs["w_norm1"], np.float32)[:, None]
    wn2 = np.asarray(inputs["w_norm2"], np.float32)[:, None]
    wq = np.asarray(inputs["wq"], np.float32) * wn1
    wk = np.asarray(inputs["wk"], np.float32) * wn1
    wv = np.asarray(inputs["wv"], np.float32) * wn1
    wo = np.asarray(inputs["wo"], np.float32)
    w1 = np.asarray(inputs["w1"], np.float32) * wn2
    w3 = np.asarray(inputs["w3"], np.float32) * wn2
    w2 = np.asarray(inputs["w2"], np.float32)

    xg = np.ascontiguousarray(x.reshape(GLOB, D))
    xT_full = np.ascontiguousarray(xg.T.astype(BF))
    perm = np.concatenate([np.arange(0, DH, 2), np.arange(1, DH, 2)])
    cosT = np.concatenate([cos.T, cos.T], axis=1)
    sinT = np.concatenate([sin.T, sin.T], axis=1)
    cscat = np.ascontiguousarray(
        np.concatenate([cosT, sinT], axis=0).astype(BF))
    sccat = np.ascontiguousarray(
        np.concatenate([sinT, cosT], axis=0).astype(BF))
    mk = np.zeros((128, 4 * 512), np.float32)
    for v in range(4):
        r = np.arange(128)[:, None] + v * 128
        q = np.arange(512)[None, :]
        mk[:, v * 512:(v + 1) * 512] = (r <= q).astype(np.float32)
    mkb = np.ascontiguousarray(mk.astype(BF))

    wo16 = np.ascontiguousarray(wo.reshape(DT, 128, D).astype(BF))
    w1b = np.ascontiguousarray(w1.astype(BF))
    w3b = np.ascontiguousarray(w3.astype(BF))
    w2fb = np.ascontiguousarray(w2.reshape(FT, 128, D).astype(BF))

    in_maps = []
    for c in range(NC):
        g = c // 2
        wqkv_c = np.empty((D, 512), np.float32)
        for hl in range(2):
            h = 2 * c + hl
            wqkv_c[:, hl * DH:(hl + 1) * DH] = wq[:, h * DH + perm]
        wqkv_c[:, 256:384] = wk[:, g * DH + perm]
        wqkv_c[:, 384:512] = wv[:, g * DH:(g + 1) * DH]
        xc_c = np.empty((TOK, D), np.float32)
        xc_c[0:HTOK, :] = xg[c * HTOK:(c + 1) * HTOK, :]
        xc_c[HTOK:TOK, :] = xg[T + c * HTOK:T + (c + 1) * HTOK, :]
        in_maps.append({
            "xT_in": xT_full,
            "xc_in": np.ascontiguousarray(xc_c.astype(BF)),
            "wqkv": np.ascontiguousarray(wqkv_c.astype(BF)),
            "wo16": wo16,
            "w1f": w1b,
            "w3f": w3b,
            "w2f": w2fb,
            "cscat": cscat,
            "sccat": sccat,
            "maskc": mkb,
        })
    return in_maps


def kernel(**inputs) -> np.ndarray:
    from concourse import bass_utils

    if "nc" not in _CACHE:
        _CACHE["nc"] = _build_program()
    nc = _CACHE["nc"]
    in_maps = _prep_inputs(inputs)
    res = bass_utils.run_bass_kernel_spmd(nc, in_maps, core_ids=list(range(NC)))
    y = np.empty((GLOB, D), np.float32)
    for c in range(NC):
        ytc = res.results[c]["yt"]
        y[c * HTOK:(c + 1) * HTOK, :] = ytc[0:HTOK, :]
        y[T + c * HTOK:T + (c + 1) * HTOK, :] = ytc[HTOK:TOK, :]
    return np.ascontiguousarray(y).astype(np.float32).reshape(B, T, D)


if __name__ == "__main__":
    import reference
    inputs = {k: np.asarray(v) for k, v in reference.setup_inputs().items()}
    out = kernel(**inputs)
    print("kernel output shape:", out.shape)

